# revision 1
# baseline (speedup 1.0000x reference)
"""PointPillars loss kernel for Trainium2 (8 NeuronCores, data parallel over batch).

Strategy
--------
The loss decomposes so that only cls_pred (24 MB) needs a bulk pass:

  f0(x) = 0.75 * sigmoid(x)^2 * softplus(x)        (focal term at target=0)
  f1(x) = 0.25 * (1-sigmoid(x))^2 * softplus(-x)   (focal term at target=1)

  cls_sum = sum_all f0(cls_pred) - sum_{window cells} f0 * wvalid
            + sum_{pos} f1(center)                 (f0(center) terms cancel)
  vm_cnt  = B*3*H*W - (#window instances - #valid boxes)
  reg/dir losses touch reg_pred/dir_pred only at the <=64 box cells per
  sample, fetched with indirect-DMA gathers.

softplus is not in any TRN2 activation table; we use
softplus(x) = -ln(1 - sigmoid(x)), exact for |x| < ~15 (inputs are ~N(0,1)).
All Sigmoid activations are issued before all Ln activations so the ACT
table is switched only twice (plus once for Sin).

Each of the 8 cores processes 2 samples: a bulk f0 reduction over its
[2,3,250,500] cls slice plus per-box (128 lanes) sparse math with three
indirect gathers.  Per-core partial sums [128,8] are combined on host
(trivial final divisions, per the all-reduce-of-(sum,count) recipe).
"""

import numpy as np

B, H, W, N = 16, 250, 500, 64
HW = H * W
NCORES = 8
BL = B // NCORES            # samples per core = 2
LANES = BL * N              # 128 boxes per core = partition dim
CLS_SZ = BL * 3 * HW        # 750000
REG_SZ = BL * 7 * HW        # 1750000
DIR_SZ = BL * 2 * HW        # 500000
BULK_P = 128
NCHUNK = 6
CF = 1024                   # per-chunk free size
BULK_F = NCHUNK * CF        # 6144; BULK_P*BULK_F = 786432 >= CLS_SZ
PAD_SZ = BULK_P * BULK_F    # host pads cls with -30.0 (f0(-30) == 0 exactly)
NSQ_ACT = 1                 # chunks whose sigma^2 runs on ACT (engine balance)
PI2 = float(np.float32(np.pi / 2))

_prog_cache = {}
_last_results = None  # BassKernelResults from the most recent run (for profiling)


def _build_program():
    import os
    import concourse.bacc as bacc
    import concourse.tile as tile
    from concourse import bass, mybir

    DO_BULK = os.environ.get("PP_BULK", "1") == "1"
    DO_BOX = os.environ.get("PP_BOX", "1") == "1"
    DO_GATHER = os.environ.get("PP_GATHER", "1") == "1"
    LAYOUT = os.environ.get("PP_LAYOUT", "col")      # col | contig
    GTQ = os.environ.get("PP_GTQ", "sync")           # scalar | sync
    nsq_act = int(os.environ.get("PP_NSQ", str(NSQ_ACT)))
    nchunk = int(os.environ.get("PP_NCHUNK", str(NCHUNK)))
    cf = BULK_F // nchunk
    assert cf % 512 == 0 and cf * nchunk == BULK_F

    f32 = mybir.dt.float32
    bf16 = mybir.dt.bfloat16
    f16 = mybir.dt.float16
    i32 = mybir.dt.int32
    A = mybir.AluOpType
    ACT = mybir.ActivationFunctionType
    X = mybir.AxisListType.X

    nc = bacc.Bacc(
        "TRN2",
        target_bir_lowering=False,
        debug=False,
        enable_asserts=False,
        num_devices=NCORES,
    )

    cls_t = nc.dram_tensor("cls", [PAD_SZ], f32, kind="ExternalInput").ap()
    reg_t = nc.dram_tensor("reg", [REG_SZ], f32, kind="ExternalInput").ap()
    dir_t = nc.dram_tensor("dirp", [DIR_SZ], f32, kind="ExternalInput").ap()
    gt_t = nc.dram_tensor("gt", [LANES, 8], f32, kind="ExternalInput").ap()
    cst_t = nc.dram_tensor("cst", [LANES, 28], f32, kind="ExternalInput").ap()
    out_t = nc.dram_tensor("part", [128, 8], f32, kind="ExternalOutput").ap()

    with tile.TileContext(nc) as tc:
        with (
            tc.tile_pool(name="bulk", bufs=4) as lp,
            tc.tile_pool(name="bsig", bufs=1) as sp_,
            tc.tile_pool(name="bln", bufs=4) as vp_,
            tc.tile_pool(name="box", bufs=1) as bx,
            tc.tile_pool(name="psum", bufs=1, space="PSUM") as pp_,
        ):
            V = nc.vector
            S = nc.scalar

            # output partials tile
            outt = bx.tile([128, 8], f32)
            V.memset(outt[:], 0.0)

            SIGMAX = float(np.float32(1.0 - 2.0 ** -20))

            if DO_BOX:
                # ------------- box inputs & offsets (DVE, no ACT) ----------
                # ACT's HWDGE queue keeps the SP queue free for the bulk
                # cls chunk stream.
                gtq = nc.scalar if GTQ == "scalar" else nc.sync
                gtt = bx.tile([LANES, 8], f32)
                gtq.dma_start(gtt[:], gt_t[:])
                cst = bx.tile([LANES, 28], f32)
                gtq.dma_start(cst[:], cst_t[:])

                xg = gtt[:, 0:1]
                yg = gtt[:, 1:2]
                zg = gtt[:, 2:3]
                rg = gtt[:, 6:7]
                cg = gtt[:, 7:8]
                bcol = cst[:, 0:1]
                woy = cst[:, 1:10]
                wox = cst[:, 10:19]
                ch7 = cst[:, 19:26]
                ch2 = cst[:, 26:28]

                # grid coords (floor(x*2.5) == floor(x/0.4) verified for f32).
                # floor(v) = int(v) - (float(int(v)) > v): exact for any
                # cast rounding mode (no mod op in the TensorScalar ISA).
                def floor_(src, name):
                    vf = bx.tile([LANES, 1], f32, tag=name + "f")
                    vi = bx.tile([LANES, 1], i32, tag=name + "i")
                    V.tensor_copy(vi[:], src)
                    vr = bx.tile([LANES, 1], f32, tag=name + "r")
                    V.tensor_copy(vr[:], vi[:])
                    adj = bx.tile([LANES, 1], f32, tag=name + "a")
                    V.tensor_tensor(adj[:], vr[:], src, A.is_gt)
                    V.tensor_sub(vf[:], vr[:], adj[:])
                    return vf

                xs = bx.tile([LANES, 1], f32)
                V.tensor_scalar_mul(xs[:], xg, 2.5)
                ys = bx.tile([LANES, 1], f32)
                V.tensor_scalar(ys[:], yg, 50.0, 2.5, A.add, A.mult)
                gxf = floor_(xs[:], "gx")
                gyf = floor_(ys[:], "gy")

                # valid mask
                vld = bx.tile([LANES, 1], f32)
                V.tensor_single_scalar(vld[:], cg, 0.0, A.is_equal)
                tmpm = bx.tile([LANES, 1], f32)
                for src, thr, op in (
                    (xg, 0.0, A.is_ge),
                    (xg, 200.0, A.is_lt),
                    (yg, -50.0, A.is_ge),
                    (yg, 50.0, A.is_lt),
                    (gxf[:], float(W), A.is_lt),
                    (gyf[:], float(H), A.is_lt),
                ):
                    V.tensor_single_scalar(tmpm[:], src, thr, op)
                    V.tensor_mul(vld[:], vld[:], tmpm[:])

                # cell id and per-sample base offsets
                cell = bx.tile([LANES, 1], f32)
                V.tensor_scalar_mul(cell[:], gyf[:], float(W))
                V.tensor_add(cell[:], cell[:], gxf[:])
                b3 = bx.tile([LANES, 1], f32)
                V.tensor_scalar_mul(b3[:], bcol, float(3 * HW))
                b7 = bx.tile([LANES, 1], f32)
                V.tensor_scalar_mul(b7[:], bcol, float(7 * HW))
                b2 = bx.tile([LANES, 1], f32)
                V.tensor_scalar_mul(b2[:], bcol, float(2 * HW))

                # 3x3 window around each center
                gy2 = bx.tile([LANES, 9], f32)
                V.tensor_single_scalar(gy2[:], woy, gyf[:], A.add)
                gx2 = bx.tile([LANES, 9], f32)
                V.tensor_single_scalar(gx2[:], wox, gxf[:], A.add)
                wv = bx.tile([LANES, 9], f32)
                V.tensor_single_scalar(wv[:], gy2[:], 0.0, A.is_ge)
                wm = bx.tile([LANES, 9], f32)
                V.tensor_single_scalar(wm[:], gy2[:], float(H), A.is_lt)
                V.tensor_mul(wv[:], wv[:], wm[:])
                V.tensor_single_scalar(wm[:], gx2[:], 0.0, A.is_ge)
                V.tensor_mul(wv[:], wv[:], wm[:])
                V.tensor_single_scalar(wm[:], gx2[:], float(W), A.is_lt)
                V.tensor_mul(wv[:], wv[:], wm[:])
                V.tensor_single_scalar(wv[:], wv[:], vld[:], A.mult)

                # cls-channel-0 flat offsets for the 9 window cells.
                # HW indirect DMA uses ONE index per partition and reads
                # contiguous elements, so clamp so start+2 stays in bounds
                # (columns 0,3,6 are the row starts at gx-1).
                cw = bx.tile([LANES, 9], f32)
                V.tensor_scalar_mul(cw[:], gy2[:], float(W))
                V.tensor_add(cw[:], cw[:], gx2[:])
                V.tensor_single_scalar(cw[:], cw[:], b3[:], A.add)
                V.tensor_scalar(cw[:], cw[:], 0.0, float(CLS_SZ - 3), A.max, A.min)
                cwi = bx.tile([LANES, 9], i32)
                V.tensor_copy(cwi[:], cw[:])

                # reg / dir gather offsets
                cb7 = bx.tile([LANES, 1], f32)
                V.tensor_add(cb7[:], cell[:], b7[:])
                roff = bx.tile([LANES, 7], f32)
                V.tensor_single_scalar(roff[:], ch7, cb7[:], A.add)
                V.tensor_scalar(roff[:], roff[:], 0.0, float(REG_SZ - 1), A.max, A.min)
                roffi = bx.tile([LANES, 7], i32)
                V.tensor_copy(roffi[:], roff[:])

                cb2 = bx.tile([LANES, 1], f32)
                V.tensor_add(cb2[:], cell[:], b2[:])
                doff = bx.tile([LANES, 2], f32)
                V.tensor_single_scalar(doff[:], ch2, cb2[:], A.add)
                V.tensor_scalar(doff[:], doff[:], 0.0, float(DIR_SZ - 1), A.max, A.min)
                doffi = bx.tile([LANES, 2], i32)
                V.tensor_copy(doffi[:], doff[:])

                winv = bx.tile([LANES, 9], f32)
                regv = bx.tile([LANES, 7], f32)
                dirv = bx.tile([LANES, 2], f32)
                if DO_GATHER:
                    # indirect gathers: HW semantics = one index per
                    # partition (first element of the offset AP row), D
                    # contiguous elements into that partition's dest row.
                    cls2d = cls_t.rearrange("(a b) -> a b", b=1)
                    reg2d = reg_t.rearrange("(a b) -> a b", b=1)
                    dir2d = dir_t.rearrange("(a b) -> a b", b=1)
                    for k in range(3):      # window rows gy-1, gy, gy+1
                        nc.gpsimd.indirect_dma_start(
                            out=winv[:, 3 * k:3 * k + 3], out_offset=None,
                            in_=cls2d,
                            in_offset=bass.IndirectOffsetOnAxis(
                                ap=cwi[:, 3 * k:3 * k + 1], axis=0),
                        )
                    for ch in range(7):
                        nc.gpsimd.indirect_dma_start(
                            out=regv[:, ch:ch + 1], out_offset=None,
                            in_=reg2d,
                            in_offset=bass.IndirectOffsetOnAxis(
                                ap=roffi[:, ch:ch + 1], axis=0),
                        )
                    for ch in range(2):
                        nc.gpsimd.indirect_dma_start(
                            out=dirv[:, ch:ch + 1], out_offset=None,
                            in_=dir2d,
                            in_offset=bass.IndirectOffsetOnAxis(
                                ap=doffi[:, ch:ch + 1], axis=0),
                        )
                else:
                    V.memset(winv[:], 0.1)
                    V.memset(regv[:], 0.2)
                    V.memset(dirv[:], 0.3)

                # ============ PHASE A (box): Sigmoids ============
                sgw = bx.tile([LANES, 9], f32)
                S.activation(sgw[:], winv[:], ACT.Sigmoid)
                sgd = bx.tile([LANES, 2], f32)
                S.activation(sgd[:], dirv[:], ACT.Sigmoid)

            if DO_BULK:
                CHUNK_SZ = BULK_P * cf
                clsv = cls_t.rearrange("(p f) -> p f", p=BULK_P)
                sgs = []
                sqs = []
                for c in range(nchunk):
                    xt = lp.tile([BULK_P, cf], f32, tag="x")
                    if LAYOUT == "contig":
                        chunk = cls_t[c * CHUNK_SZ:(c + 1) * CHUNK_SZ].rearrange(
                            "(p f) -> p f", p=BULK_P)
                    else:
                        chunk = clsv[:, c * cf:(c + 1) * cf]
                    nc.sync.dma_start(xt[:], chunk)
                    sg = sp_.tile([BULK_P, cf], f32, tag=f"sg{c}")
                    S.activation(sg[:], xt[:], ACT.Sigmoid)
                    sgs.append(sg)
                    if c < nsq_act:  # Square is in the sigmoid table: no switch
                        sq = sp_.tile([BULK_P, cf], f16, tag=f"sq{c}")
                        S.activation(sq[:], sg[:], ACT.Square)
                        sqs.append(sq)
                    else:
                        sqs.append(None)

            if DO_BOX:
                # ============ PHASE B (box): Ln ============
                vw = bx.tile([LANES, 9], f32)
                V.tensor_single_scalar(sgw[:], sgw[:], SIGMAX, A.min)
                S.activation(vw[:], sgw[:], ACT.Ln, scale=-1.0, bias=1.0)
                vd = bx.tile([LANES, 2], f32)
                V.tensor_single_scalar(sgd[:], sgd[:], SIGMAX, A.min)
                S.activation(vd[:], sgd[:], ACT.Ln, scale=-1.0, bias=1.0)
                lwh = bx.tile([LANES, 3], f32)
                V.tensor_single_scalar(lwh[:], gtt[:, 3:6], 1e-3, A.max)
                lnwh = bx.tile([LANES, 3], f32)
                S.activation(lnwh[:], lwh[:], ACT.Ln)

            if DO_BULK:
                # ones for the PE partition-reduction
                ones = bx.tile([BULK_P, 1], f16)
                V.memset(ones[:], 1.0)
                acc = pp_.tile([1, 512], f32)
                NMM = cf // 512
                for c in range(nchunk):
                    sg = sgs[c]
                    v = vp_.tile([BULK_P, cf], f16, tag="v")
                    S.activation(v[:], sg[:], ACT.Ln, scale=-1.0, bias=1.0)
                    if sqs[c] is None:
                        sq = vp_.tile([BULK_P, cf], f16, tag="sq")
                        V.tensor_mul(sq[:], sg[:], sg[:])
                    else:
                        sq = sqs[c]
                    prod = vp_.tile([BULK_P, cf], f16, tag="prod")
                    V.tensor_mul(prod[:], sq[:], v[:])  # = -sigma^2*sp (bf16)
                    for m in range(NMM):
                        nc.tensor.matmul(
                            acc[:], ones[:], prod[:, m * 512:(m + 1) * 512],
                            start=(c == 0 and m == 0),
                            stop=(c == nchunk - 1 and m == NMM - 1))
                accs = bx.tile([1, 512], f32)
                V.tensor_copy(accs[:], acc[:])
                red0 = bx.tile([1, 1], f32)
                V.tensor_reduce(red0[:], accs[:], axis=X, op=A.add)
                V.tensor_scalar_mul(outt[0:1, 0:1], red0[:], -0.75)

            if DO_BOX:
                # ============ PHASE C: Sin ============
                sinr = bx.tile([LANES, 1], f32)
                S.activation(sinr[:], rg, ACT.Sin)

                # ---------------- sparse cls corrections ----------------
                f0w = bx.tile([LANES, 9], f32)
                V.tensor_mul(f0w[:], sgw[:], sgw[:])
                V.tensor_mul(f0w[:], f0w[:], vw[:])       # = -sigma^2 * sp
                f0s = bx.tile([LANES, 9], f32)
                V.tensor_mul(f0s[:], f0w[:], wv[:])
                V.tensor_scalar_mul(f0s[:], f0s[:], -0.75)
                V.tensor_reduce(outt[:, 1:2], f0s[:], axis=X, op=A.add)
                V.tensor_reduce(outt[:, 3:4], wv[:], axis=X, op=A.add)
                V.tensor_copy(outt[:, 4:5], vld[:])

                # f1 at centers: 0.25*(1-sig)^2*(sp - x) * valid
                sm1 = bx.tile([LANES, 1], f32)
                V.tensor_scalar_add(sm1[:], sgw[:, 4:5], -1.0)
                V.tensor_mul(sm1[:], sm1[:], sm1[:])
                spx = bx.tile([LANES, 1], f32)
                V.tensor_add(spx[:], vw[:, 4:5], winv[:, 4:5])   # = -(sp - x)
                V.tensor_mul(sm1[:], sm1[:], spx[:])
                v25 = bx.tile([LANES, 1], f32)
                V.tensor_scalar_mul(v25[:], vld[:], -0.25)
                V.tensor_mul(outt[:, 2:3], sm1[:], v25[:])

                # ---------------- regression smooth-L1 ----------------
                regt = bx.tile([LANES, 7], f32)
                cx = bx.tile([LANES, 1], f32)
                V.tensor_scalar(cx[:], gxf[:], 0.5, 0.4, A.add, A.mult)
                dxv = bx.tile([LANES, 1], f32)
                V.tensor_sub(dxv[:], xg, cx[:])
                V.tensor_scalar_mul(regt[:, 0:1], dxv[:], 2.5)
                cy = bx.tile([LANES, 1], f32)
                V.tensor_scalar(cy[:], gyf[:], 0.5, 0.4, A.add, A.mult)
                V.tensor_scalar_add(cy[:], cy[:], -50.0)
                dyv = bx.tile([LANES, 1], f32)
                V.tensor_sub(dyv[:], yg, cy[:])
                V.tensor_scalar_mul(regt[:, 1:2], dyv[:], 2.5)
                V.tensor_copy(regt[:, 2:3], zg)
                V.tensor_copy(regt[:, 3:6], lnwh[:])
                V.tensor_copy(regt[:, 6:7], sinr[:])

                dreg = bx.tile([LANES, 7], f32)
                V.tensor_sub(dreg[:], regv[:], regt[:])
                dregn = bx.tile([LANES, 7], f32)
                V.tensor_scalar_mul(dregn[:], dreg[:], -1.0)
                V.tensor_max(dreg[:], dreg[:], dregn[:])
                mlt = bx.tile([LANES, 7], f32)
                V.tensor_single_scalar(mlt[:], dreg[:], 1.0, A.is_lt)
                qd = bx.tile([LANES, 7], f32)
                V.tensor_mul(qd[:], dreg[:], dreg[:])
                V.tensor_scalar_mul(qd[:], qd[:], 0.5)
                lin = bx.tile([LANES, 7], f32)
                V.tensor_scalar_add(lin[:], dreg[:], -0.5)
                V.tensor_sub(qd[:], qd[:], lin[:])
                V.tensor_mul(qd[:], qd[:], mlt[:])
                V.tensor_add(qd[:], qd[:], lin[:])
                red5 = bx.tile([LANES, 1], f32)
                V.tensor_reduce(red5[:], qd[:], axis=X, op=A.add)
                V.tensor_mul(outt[:, 5:6], red5[:], vld[:])

                # ---------------- direction BCE ----------------
                # sum_ch (sp(dirv) - dirv*dirt) = sum_ch -(vd + dirv*dirt)
                dirt = bx.tile([LANES, 2], f32)
                ab = bx.tile([LANES, 1], f32)
                abn = bx.tile([LANES, 1], f32)
                V.tensor_scalar_mul(abn[:], rg, -1.0)
                V.tensor_max(ab[:], rg, abn[:])
                V.tensor_single_scalar(dirt[:, 0:1], ab[:], PI2, A.is_le)
                V.tensor_single_scalar(dirt[:, 1:2], ab[:], PI2, A.is_gt)
                V.tensor_mul(dirt[:], dirt[:], dirv[:])
                V.tensor_add(dirt[:], dirt[:], vd[:])
                red6 = bx.tile([LANES, 1], f32)
                V.tensor_reduce(red6[:], dirt[:], axis=X, op=A.add)
                vneg = bx.tile([LANES, 1], f32)
                V.tensor_scalar_mul(vneg[:], vld[:], -1.0)
                V.tensor_mul(outt[:, 6:7], red6[:], vneg[:])

            nc.sync.dma_start(out_t[:], outt[:])

    nc.compile()
    return nc


def _lane_consts():
    cst = np.zeros((LANES, 28), np.float32)
    cst[:, 0] = np.repeat(np.arange(BL), N)          # sample index within core
    oy, ox = np.meshgrid([-1, 0, 1], [-1, 0, 1], indexing="ij")
    cst[:, 1:10] = oy.ravel()[None, :]
    cst[:, 10:19] = ox.ravel()[None, :]
    cst[:, 19:26] = (np.arange(7) * HW)[None, :]
    cst[:, 26:28] = (np.arange(2) * HW)[None, :]
    return cst


def kernel(cls_pred, reg_pred, dir_pred, gt_boxes, batch_size=None):
    from concourse import bass_utils

    cls_pred = np.ascontiguousarray(cls_pred, dtype=np.float32)
    reg_pred = np.ascontiguousarray(reg_pred, dtype=np.float32)
    dir_pred = np.ascontiguousarray(dir_pred, dtype=np.float32)
    gt_boxes = np.ascontiguousarray(gt_boxes, dtype=np.float32)

    if "nc" not in _prog_cache:
        _prog_cache["nc"] = _build_program()
    nc = _prog_cache["nc"]

    cst = _lane_consts()
    in_maps = []
    for c in range(NCORES):
        b0 = c * BL
        cls_pad = np.full(PAD_SZ, -30.0, np.float32)
        cls_pad[:CLS_SZ] = cls_pred[b0:b0 + BL].reshape(-1)
        in_maps.append({
            "cls": cls_pad,
            "reg": reg_pred[b0:b0 + BL].reshape(-1),
            "dirp": dir_pred[b0:b0 + BL].reshape(-1),
            "gt": gt_boxes[b0:b0 + BL].reshape(LANES, 8),
            "cst": cst,
        })

    res = bass_utils.run_bass_kernel_spmd(nc, in_maps, core_ids=list(range(NCORES)))
    global _last_results
    _last_results = res
    parts = np.stack([r["part"] for r in res.results])  # [8,128,8]
    col = parts.sum(axis=(0, 1), dtype=np.float64)

    bulk, c1, c2, wcnt, nval = col[0], col[1], col[2], col[3], col[4]
    reg_s, dir_s = col[5], col[6]

    cls_sum = bulk - c1 + c2
    vm_cnt = B * 3 * HW - (wcnt - nval)
    cls_loss = cls_sum / max(vm_cnt, 1.0)
    reg_loss = reg_s / max(7.0 * nval, 1.0)
    dir_loss = dir_s / max(2.0 * nval, 1.0)
    total = 1.0 * cls_loss + 2.0 * reg_loss + 0.2 * dir_loss
    return np.array([total, cls_loss, reg_loss, dir_loss], dtype=np.float32)

